# revision 21
# baseline (speedup 1.0000x reference)
"""nn_CrossAttention Trainium2 Bass kernel.

Sharding (8 cores): data-parallel over batch (4 samples x 2 cores) with
2-way Megatron tensor parallelism inside each pair: core = (sample, half).
Each half owns 8 of 16 attention heads (Wq cols / Wout rows) and 2048 of
4096 ff_inner channels (Wff1 cols / Wff2 rows); the tiny shared-head Wkv is
replicated.  Per-core partial outputs (attn@Wout_half + ff@Wff2_half) are
summed pairwise on the host, which also owns the final transpose (the
device computes the output feature-major).

Device kernel (per core, identical SPMD program):
  - LayerNorm token-major via bn_stats (gains folded into the weights on
    the host), then PE-transpose to feature-major.
  - One activation-table set for the whole kernel
    (natural_log_exp_and_others, forced by a post-compile rewrite of the
    InstLoadActFuncSet ids): LN rstd = exp(-0.5*ln(var+eps)), attention
    softmax exp, SwiGLU sigmoid = exp(-ln(1+exp(-g))) all on ScalarE.
  - All matmuls in bf16 with 512-wide moving operands (fp32 PSUM accum).
  - Attention computed transposed (keys/queries feature-major, sim with
    context positions on partitions) so softmax sums fold into the
    attn@v matmul as a ones-column of the [v | 1] stationary operand.
    The two heads of a pair run concurrently on disjoint PE row groups
    (tile_position (0,0)/(64,0)).
  - FF1 (SwiGLU) matmuls interleaved at matmul granularity into the
    attention loops so TensorE never head-of-line blocks on ScalarE exp.
  - Out-projection accumulates the attention and FF paths into one PSUM
    group, streaming Wout/Wff2 once (mt-outer, qc-inner).
"""
import sys

if "/opt/trn_rl_repo" not in sys.path:
    sys.path.insert(0, "/opt/trn_rl_repo")

import numpy as np

import concourse.bass as bass  # noqa: F401  (bass must import before bacc)
import concourse.mybir as mybir
import concourse.tile as tile
from concourse import bacc, bass_utils
from concourse.hw_specs import get_activation_tables

F32 = mybir.dt.float32
BF16 = mybir.dt.bfloat16
AF = mybir.ActivationFunctionType
ALU = mybir.AluOpType

P = 128
B = 4           # batch
NTOK = 1024     # query tokens per sample
NCTX = 1024     # context tokens per sample
DIM = 1024
DH = 64         # head dim
HC = 8          # heads per core (16 total / 2-way TP)
QF = HC * DH    # 512 per-core q features
FFC = 2048      # per-core ff_inner channels
EPS = 1e-5
SCALE = DH ** -0.5

TT = NTOK // P   # 8 token tiles
KT = DIM // P    # 8 contraction tiles over dim
QC = NTOK // 512  # 2 moving-operand chunks of 512 tokens

_CACHED = {}


def _collapse_act_table_loads(nc):
    """All activation funcs used (ln/exp/copy) live in one table set;
    rewrite the compiler-inserted per-function loads into a single load
    of natural_log_exp_and_others at the first site."""
    tables = get_activation_tables(nc.m.arch)
    names = list(tables.keys())
    combined = names.index("natural_log_exp_and_others")
    allowed = tables["natural_log_exp_and_others"]
    used = set()
    for b in nc.main_func.blocks:
        for i in b.instructions:
            if isinstance(i, mybir.InstActivation):
                used.add(i.func)
    assert used <= allowed, f"activation funcs {used - allowed} not in combined set"
    for b in nc.main_func.blocks:
        kept_one = False
        keep = []
        for i in b.instructions:
            if isinstance(i, mybir.InstLoadActFuncSet):
                assert i.sync_info is None
                if not kept_one:
                    i.act_func_set_id = combined
                    kept_one = True
                    keep.append(i)
            else:
                keep.append(i)
        b.instructions[:] = keep


def _build(with_bias: bool):
    nc = bacc.Bacc("TRN2", target_bir_lowering=False, debug=False)

    x_d = nc.dram_tensor("x", [NTOK, DIM], F32, kind="ExternalInput").ap()
    c_d = nc.dram_tensor("ctx", [NCTX, DIM], F32, kind="ExternalInput").ap()
    wq_d = nc.dram_tensor("wq", [DIM, QF], BF16, kind="ExternalInput").ap()
    wkv_d = nc.dram_tensor("wkv", [DIM, 2 * DH], BF16, kind="ExternalInput").ap()
    wout_d = nc.dram_tensor("wout", [QF, DIM], BF16, kind="ExternalInput").ap()
    wff1_d = nc.dram_tensor("wff1", [DIM, 2 * FFC], BF16, kind="ExternalInput").ap()
    wff2_d = nc.dram_tensor("wff2", [FFC, DIM], BF16, kind="ExternalInput").ap()
    eyer_d = nc.dram_tensor("eyer", [P, P], BF16, kind="ExternalInput").ap()
    ones_d = nc.dram_tensor("onesd", [P, 1], BF16, kind="ExternalInput").ap()
    if with_bias:
        bq_d = nc.dram_tensor("bq", [1, QF], F32, kind="ExternalInput").ap()
        bkv_d = nc.dram_tensor("bkv", [1, 2 * DH], F32, kind="ExternalInput").ap()
        bff1_d = nc.dram_tensor("bff1", [1, 2 * FFC], F32, kind="ExternalInput").ap()
    out_d = nc.dram_tensor("out", [DIM, NTOK], F32, kind="ExternalOutput").ap()

    # dram views tiled for lhsT streaming: [p, ktile, cols]
    wq_v = wq_d.rearrange("(ko p) c -> p ko c", p=P)
    wkv_v = wkv_d.rearrange("(ko p) c -> p ko c", p=P)
    wout_v = wout_d.rearrange("(ko p) c -> p ko c", p=P)
    wff1_v = wff1_d.rearrange("(ko p) c -> p ko c", p=P)
    wff2_v = wff2_d.rearrange("(ko p) c -> p ko c", p=P)

    with tile.TileContext(nc) as tc:
        with (
            tc.tile_pool(name="consts", bufs=1) as consts,
            tc.tile_pool(name="lnx", bufs=4) as lnxp,
            tc.tile_pool(name="ln", bufs=3) as lnp,
            tc.tile_pool(name="small", bufs=2) as smallp,
            tc.tile_pool(name="small1", bufs=1) as smallp1,
            tc.tile_pool(name="resid", bufs=1) as resid,
            tc.tile_pool(name="big", bufs=1) as bigp,
            tc.tile_pool(name="wff", bufs=4) as wffp,
            tc.tile_pool(name="wop", bufs=2) as wop,
            tc.tile_pool(name="attn", bufs=2) as attnp,
            tc.tile_pool(name="pm", bufs=3, space="PSUM") as pmp,
            tc.tile_pool(name="po", bufs=2, space="PSUM") as pop,
            tc.tile_pool(name="pf", bufs=3, space="PSUM") as pfp,
        ):
            identr = consts.tile([P, P], BF16)
            nc.sync.dma_start(identr[:], eyer_d[:])
            eps_t = consts.tile([P, 1], F32)
            nc.vector.memset(eps_t[:], EPS)
            one_t = consts.tile([P, 1], F32)
            nc.vector.memset(one_t[:], 1.0)
            # x tile preloads go out on the sync queue before any weight
            # traffic so the LN pipeline starts immediately
            xts = []
            for tt in range(4):
                xt = lnxp.tile([P, DIM], F32, tag="xpre", name="xpre")
                nc.sync.dma_start(xt[:], x_d[tt * P:(tt + 1) * P, :])
                xts.append(xt)
            # HAM warmup: keep the PE busy through the LN head so the first
            # real matmuls run at the full 2.4 GHz clock
            for _ in range(36):
                pw = pmp.tile([P, 512], F32, tag="pm", name="warm")
                nc.tensor.matmul(
                    pw[:, 0:P], identr[:], identr[:], start=True, stop=True
                )
            if with_bias:
                bq_t = consts.tile([P, QF // P], F32)
                nc.sync.dma_start(bq_t[:], bq_d.rearrange("o (fo p) -> p (o fo)", p=P))
                bkv_t = consts.tile([P, 1], F32)
                nc.sync.dma_start(bkv_t[:], bkv_d.rearrange("o (fo p) -> p (o fo)", p=P))
                bff1_t = consts.tile([P, (2 * FFC) // P], F32)
                nc.sync.dma_start(
                    bff1_t[:], bff1_d.rearrange("o (fo p) -> p (o fo)", p=P)
                )

            # persistent activations / weights
            xn_F = resid.tile([P, KT, NTOK], BF16)      # normalized x, feature-major
            cn_F = bigp.tile([P, KT, NCTX], BF16, tag="bigc", name="cn_F")
            qT = resid.tile([P, QF // P, NTOK], BF16)   # queries, feature-major
            kv_sb = resid.tile([P, NCTX], BF16)         # rows 0:64 v, 64:128 k
            kdup = resid.tile([P, NCTX], BF16)          # rows 0:64 = copy of k
            v_aug = resid.tile([P, NCTX // P, DH + 1], BF16)  # [j-in-tile, jt, v|1]
            attn_outT = resid.tile([P, QF // P, NTOK], BF16)
            ff_sc = [
                bigp.tile([P, FFC // P, 512], BF16, tag="big", name="ff_sc0"),
                bigp.tile([P, FFC // P, 512], BF16, tag="big2", name="ff_sc1"),
            ]
            wq_all = resid.tile([P, KT, QF], BF16)
            wkv_t = resid.tile([P, KT, 2 * DH], BF16)
            nc.sync.dma_start(wq_all[:], wq_v[:])
            nc.sync.dma_start(wkv_t[:], wkv_v[:])

            def layernorm_iter(src_dram, dst_fmajor, tt, xt=None):
                if xt is None:
                    xt = lnp.tile([P, DIM], F32, tag="xt", name="xt")
                    nc.sync.dma_start(xt[:], src_dram[tt * P:(tt + 1) * P, :])
                st = lnp.tile([P, 2, nc.vector.BN_STATS_DIM], F32, tag="lnst")
                xv = xt.rearrange("p (s f) -> p s f", s=2)
                nc.vector.bn_stats(st[:, 0, :], xv[:, 0, :])
                nc.vector.bn_stats(st[:, 1, :], xv[:, 1, :])
                mv = lnp.tile([P, nc.vector.BN_AGGR_DIM], F32, tag="lnmv")
                nc.vector.bn_aggr(mv[:], st[:])
                # rstd = exp(-0.5 * ln(var + eps)) — stays in the ln/exp set
                lv = lnp.tile([P, 1], F32, tag="lnlv")
                nc.scalar.activation(
                    out=lv[:], in_=mv[:, 1:2], func=AF.Ln, bias=eps_t[:]
                )
                rstd = lnp.tile([P, 1], F32, tag="lnrs")
                nc.scalar.activation(out=rstd[:], in_=lv[:], func=AF.Exp, scale=-0.5)
                xh = lnp.tile([P, DIM], BF16, tag="lnh")
                nc.vector.tensor_scalar(
                    out=xh[:], in0=xt[:], scalar1=mv[:, 0:1], scalar2=rstd[:],
                    op0=ALU.subtract, op1=ALU.mult,
                )
                for dt_ in range(KT):
                    pt = pmp.tile([P, 512], BF16, tag="pm", name="pt")
                    nc.tensor.transpose(
                        pt[:, 0:P], xh[:, dt_ * P:(dt_ + 1) * P], identr[:]
                    )
                    if dt_ % 2 == 0:
                        nc.vector.tensor_copy(
                            dst_fmajor[:, dt_, tt * P:(tt + 1) * P], pt[:, 0:P]
                        )
                    else:
                        nc.scalar.activation(
                            out=dst_fmajor[:, dt_, tt * P:(tt + 1) * P],
                            in_=pt[:, 0:P], func=AF.Copy,
                        )

            def q_chunk(ft, qc):
                pq = pmp.tile([P, 512], F32, tag="pm")
                for k in range(KT):
                    nc.tensor.matmul(
                        pq[:], wq_all[:, k, ft * P:(ft + 1) * P],
                        xn_F[:, k, qc * 512:(qc + 1) * 512],
                        start=(k == 0), stop=(k == KT - 1),
                    )
                if with_bias:
                    nc.vector.tensor_scalar_add(
                        out=qT[:, ft, qc * 512:(qc + 1) * 512],
                        in0=pq[:], scalar1=bq_t[:, ft:ft + 1],
                    )
                else:
                    nc.vector.tensor_copy(qT[:, ft, qc * 512:(qc + 1) * 512], pq[:])

            # ---- ff1 (SwiGLU) machinery: a chain of matmul-granular steps ----
            ff_sched = [(0, i) for i in range(FFC // P)] + \
                       [(1, i) for i in range(FFC // P)]
            ff_w = {}

            def ff_prefetch(idx):
                if idx >= len(ff_sched) or idx in ff_w:
                    return
                _, i = ff_sched[idx]
                # iters consumed in phases B/C load on the (otherwise idle)
                # scalar HWDGE queue so input loads own the sync queue; the
                # first two stay on sync so their dispatches don't delay the
                # phase-A LN activations queued behind them on ScalarE
                eng = nc.scalar if 2 <= idx < 8 else nc.sync
                wv = wffp.tile([P, KT, P], BF16, tag="ffw", name="ffwv")
                eng.dma_start(wv[:], wff1_v[:, :, i * P:(i + 1) * P])
                wg = wffp.tile([P, KT, P], BF16, tag="ffw", name="ffwg")
                eng.dma_start(
                    wg[:], wff1_v[:, :, FFC + i * P:FFC + (i + 1) * P]
                )
                ff_w[idx] = (wv, wg)

            def ff_ops(idx):
                qc, i = ff_sched[idx]
                wv, wg = ff_w.pop(idx)
                ff_prefetch(idx + 1)
                pv = pfp.tile([P, 512], F32, tag="pf")
                pg = pfp.tile([P, 512], F32, tag="pf")
                for k in range(KT):
                    nc.tensor.matmul(
                        pv[:], wv[:, k, :], xn_F[:, k, qc * 512:(qc + 1) * 512],
                        start=(k == 0), stop=(k == KT - 1),
                    )
                    yield
                for k in range(KT):
                    nc.tensor.matmul(
                        pg[:], wg[:, k, :], xn_F[:, k, qc * 512:(qc + 1) * 512],
                        start=(k == 0), stop=(k == KT - 1),
                    )
                    yield
                # silu(g)*v = g*sigmoid(g)*v; sigmoid via exp/ln/exp only
                if with_bias:
                    gsl = smallp.tile([P, 512], F32, tag="silg")
                    nc.vector.tensor_scalar_add(
                        out=gsl[:], in0=pg[:],
                        scalar1=bff1_t[:, FFC // P + i:FFC // P + i + 1],
                    )
                    vsl = smallp.tile([P, 512], F32, tag="silv")
                    nc.vector.tensor_scalar_add(
                        out=vsl[:], in0=pv[:], scalar1=bff1_t[:, i:i + 1]
                    )
                else:
                    gsl, vsl = pg, pv
                e = smallp.tile([P, 512], F32, tag="sile")
                nc.scalar.activation(out=e[:], in_=gsl[:], func=AF.Exp, scale=-1.0)
                w = smallp.tile([P, 512], F32, tag="silw")
                nc.scalar.activation(out=w[:], in_=e[:], func=AF.Ln, bias=one_t[:])
                sg = smallp.tile([P, 512], F32, tag="sils")
                nc.scalar.activation(out=sg[:], in_=w[:], func=AF.Exp, scale=-1.0)
                m = smallp.tile([P, 512], F32, tag="silm")
                nc.vector.tensor_tensor(m[:], gsl[:], sg[:], ALU.mult)
                nc.vector.tensor_tensor(ff_sc[qc][:, i, :], m[:], vsl[:], ALU.mult)

            def _ff_chain():
                for idx in range(len(ff_sched)):
                    yield from ff_ops(idx)

            ff_chain = _ff_chain()

            def ff_step(n):
                for _ in range(n):
                    next(ff_chain, None)

            # ---- phase A: LN(x) head + weight prefetch ----
            ff_prefetch(0)
            for tt in range(4):
                layernorm_iter(x_d, xn_F, tt, xt=xts[tt])

            # ---- phase B: q qc=0 + early ff1 while LN(x) tail runs on DVE ----
            ctx_pre = {}

            def ctx_prefetch(ct):
                if ct >= 2 * (QF // P) or ct in ctx_pre:
                    return
                xt = lnp.tile([P, DIM], F32, tag="xt", name="cpre")
                nc.sync.dma_start(xt[:], c_d[ct * P:(ct + 1) * P, :])
                ctx_pre[ct] = xt

            # first two ctx tiles ride the freed phase-A preload slots so
            # phase C's LN chain starts without waiting on the DMA queue
            for ct in range(2):
                cxt = lnxp.tile([P, DIM], F32, tag="xpre", name="cpre")
                nc.scalar.dma_start(cxt[:], c_d[ct * P:(ct + 1) * P, :])
                ctx_pre[ct] = cxt
            for ft in range(QF // P):
                layernorm_iter(x_d, xn_F, 4 + ft)
                q_chunk(ft, 0)
                ff_step(8)

            # ---- phase C: q qc=1 + LN(ctx) + early ff1 ----
            for ft in range(QF // P):
                ctx_prefetch(2 * ft + 2)
                layernorm_iter(c_d, cn_F, 2 * ft, xt=ctx_pre.pop(2 * ft))
                q_chunk(ft, 1)
                ctx_prefetch(2 * ft + 3)
                layernorm_iter(c_d, cn_F, 2 * ft + 1, xt=ctx_pre.pop(2 * ft + 1))
                ff_step(16)

            # ---- phase D: kv = cn @ Wkv + v transpose ----
            for jc in range(NCTX // 512):
                pkv = pmp.tile([P, 512], F32, tag="pm")
                for k in range(KT):
                    nc.tensor.matmul(
                        pkv[0:2 * DH, :], wkv_t[:, k, :],
                        cn_F[:, k, jc * 512:(jc + 1) * 512],
                        start=(k == 0), stop=(k == KT - 1),
                    )
                if with_bias:
                    nc.vector.tensor_scalar_add(
                        out=kv_sb[:, jc * 512:(jc + 1) * 512],
                        in0=pkv[0:2 * DH, :], scalar1=bkv_t[:],
                    )
                else:
                    nc.vector.tensor_copy(
                        kv_sb[:, jc * 512:(jc + 1) * 512], pkv[0:2 * DH, :]
                    )
                ff_step(8)
            # k lives at partitions 64:128; duplicate at 0:64 for even heads
            nc.sync.dma_start(kdup[0:DH, :], kv_sb[DH:2 * DH, :])
            # v (partitions 0:64) transposed to token-major with a ones column
            for jt in range(NCTX // P):
                pv_ = pmp.tile([P, 512], BF16, tag="pm")
                nc.tensor.transpose(
                    pv_[:, 0:DH], kv_sb[0:DH, jt * P:(jt + 1) * P],
                    identr[0:DH, 0:DH],
                )
                nc.vector.tensor_copy(v_aug[:, jt, 0:DH], pv_[:, 0:DH])
                ff_step(1)
            nc.sync.dma_start(
                v_aug[:, :, DH:DH + 1],
                bass.AP(tensor=ones_d.tensor, offset=0,
                        ap=[list(ones_d.ap[0]), [0, NCTX // P], list(ones_d.ap[1])]),
            )

            # ---- phase E: attention pairs with ff1 interleaved per-matmul ----
            def attn_pair(ft, qc):
                expT = attnp.tile([P, NCTX // P, 2, 512], BF16, tag="expT")
                qsl = [
                    qT[0:DH, ft, qc * 512:(qc + 1) * 512],
                    qT[DH:2 * DH, ft, qc * 512:(qc + 1) * 512],
                ]
                for jt in range(NCTX // P):
                    ps0 = pmp.tile([P, 512], F32, tag="pm")
                    ps1 = pmp.tile([P, 512], F32, tag="pm")
                    nc.tensor.matmul(
                        ps0[:], kdup[0:DH, jt * P:(jt + 1) * P], qsl[0],
                        start=True, stop=True, tile_position=(0, 0),
                    )
                    nc.tensor.matmul(
                        ps1[:], kv_sb[DH:2 * DH, jt * P:(jt + 1) * P], qsl[1],
                        start=True, stop=True, tile_position=(64, 0),
                    )
                    nc.scalar.activation(out=expT[:, jt, 0, :], in_=ps0[:], func=AF.Exp)
                    nc.scalar.activation(out=expT[:, jt, 1, :], in_=ps1[:], func=AF.Exp)
                    ff_step(4)
                po = [pop.tile([P, 512], F32, tag="po", name=f"po{e}") for e in range(2)]
                for jt in range(NCTX // P):
                    for e in range(2):
                        nc.tensor.matmul(
                            po[e][0:DH + 1, :], v_aug[:, jt, :], expT[:, jt, e, :],
                            start=(jt == 0), stop=(jt == NCTX // P - 1),
                        )
                    ff_step(2)
                for e in range(2):
                    rec = smallp1.tile([P, 512], F32, tag="rec")
                    # move the sums row (psum partition 64) to partition 0
                    nc.vector.tensor_copy(rec[DH:DH + 1, :], po[e][DH:DH + 1, :])
                    nc.sync.dma_start(rec[0:1, :], rec[DH:DH + 1, :])
                    nc.vector.reciprocal_approx_fast(out=rec[0:1, :], in_=rec[0:1, :])
                    rb = smallp1.tile([DH, 512], F32, tag="rb")
                    nc.gpsimd.partition_broadcast(rb[:], rec[0:1, :])
                    if e == 0:
                        nc.vector.tensor_tensor(
                            attn_outT[0:DH, ft, qc * 512:(qc + 1) * 512],
                            po[e][0:DH, :], rb[:], ALU.mult,
                        )
                    else:
                        stg = smallp1.tile([DH, 512], BF16, tag="stg")
                        nc.vector.tensor_tensor(stg[:], po[e][0:DH, :], rb[:], ALU.mult)
                        nc.sync.dma_start(
                            attn_outT[DH:2 * DH, ft, qc * 512:(qc + 1) * 512], stg[:]
                        )

            for ft in range(QF // P):
                for qc in range(QC):
                    attn_pair(ft, qc)

            # ---- phase G setup + phase F: drain remaining ff1 ----
            op_w = {}

            def op_prefetch(mt):
                if mt >= DIM // P or mt in op_w:
                    return
                wo = wop.tile([P, QF // P, P], BF16, tag="wo", name="wo_t")
                nc.sync.dma_start(wo[:], wout_v[:, :, mt * P:(mt + 1) * P])
                wf2 = wop.tile([P, FFC // P, P], BF16, tag="wf2", name="wf2_t")
                nc.sync.dma_start(wf2[:], wff2_v[:, :, mt * P:(mt + 1) * P])
                op_w[mt] = (wo, wf2)

            op_prefetch(0)
            op_prefetch(1)
            ff_step(len(ff_sched) * 2 * KT)
            for mt in range(DIM // P):
                wo, wf2 = op_w.pop(mt)
                op_prefetch(mt + 1)
                for qc in range(QC):
                    pout = pmp.tile([P, 512], F32, tag="pm")
                    for k in range(QF // P):
                        nc.tensor.matmul(
                            pout[:], wo[:, k, :],
                            attn_outT[:, k, qc * 512:(qc + 1) * 512],
                            start=(k == 0), stop=False,
                        )
                    for k in range(FFC // P):
                        nc.tensor.matmul(
                            pout[:], wf2[:, k, :], ff_sc[qc][:, k, :],
                            start=False, stop=(k == FFC // P - 1),
                        )
                    ot = smallp.tile([P, 512], F32, tag="ot")
                    nc.vector.tensor_copy(ot[:], pout[:])
                    nc.sync.dma_start(
                        out_d[mt * P:(mt + 1) * P, qc * 512:(qc + 1) * 512], ot[:]
                    )

    nc.compile()
    _collapse_act_table_loads(nc)
    return nc


def _get_program(with_bias: bool):
    key = ("nc", with_bias)
    if key not in _CACHED:
        _CACHED[key] = _build(with_bias)
    return _CACHED[key]


def kernel(x, context, ln_x_g, ln_x_b, ln_c_g, ln_c_b, Wq, Wkv, Wout, Wff1, Wff2):
    x = np.asarray(x, np.float32)
    context = np.asarray(context, np.float32)
    ln_x_g = np.asarray(ln_x_g, np.float32)
    ln_x_b = np.asarray(ln_x_b, np.float32)
    ln_c_g = np.asarray(ln_c_g, np.float32)
    ln_c_b = np.asarray(ln_c_b, np.float32)
    Wq = np.asarray(Wq, np.float32)
    Wkv = np.asarray(Wkv, np.float32)
    Wout = np.asarray(Wout, np.float32)
    Wff1 = np.asarray(Wff1, np.float32)
    Wff2 = np.asarray(Wff2, np.float32)

    # fold LN gains (and the attention scale) into the weights
    wq_eff = (ln_x_g[:, None] * Wq) * SCALE          # [1024, 1024]
    wkv_eff = ln_c_g[:, None] * Wkv                  # [1024, 128]
    # device kv layout: v at features 0:64, k at 64:128
    wkv_eff = np.concatenate([wkv_eff[:, DH:], wkv_eff[:, :DH]], axis=1)
    wff1_eff = ln_x_g[:, None] * Wff1                # [1024, 8192]
    with_bias = bool(np.any(ln_x_b != 0.0) or np.any(ln_c_b != 0.0))
    if with_bias:
        bq_eff = (ln_x_b @ Wq) * SCALE               # [1024]
        bkv_eff = ln_c_b @ Wkv                       # [128]
        bkv_eff = np.concatenate([bkv_eff[DH:], bkv_eff[:DH]])
        bff1_eff = ln_x_b @ Wff1                     # [8192]

    import ml_dtypes
    bf16 = ml_dtypes.bfloat16
    eye = np.eye(P, dtype=bf16)
    onesd = np.ones((P, 1), bf16)
    in_maps = []
    for c in range(8):
        s, t = c // 2, c % 2
        m = {
            "x": np.ascontiguousarray(x[s]),
            "ctx": np.ascontiguousarray(context[s]),
            "wq": np.ascontiguousarray(wq_eff[:, QF * t:QF * (t + 1)].astype(bf16)),
            "wkv": np.ascontiguousarray(wkv_eff.astype(bf16)),
            "wout": np.ascontiguousarray(Wout[QF * t:QF * (t + 1), :].astype(bf16)),
            "wff1": np.ascontiguousarray(np.concatenate(
                [wff1_eff[:, FFC * t:FFC * (t + 1)],
                 wff1_eff[:, 2 * FFC + FFC * t:2 * FFC + FFC * (t + 1)]],
                axis=1).astype(bf16)),
            "wff2": np.ascontiguousarray(Wff2[FFC * t:FFC * (t + 1), :].astype(bf16)),
            "eyer": eye,
            "onesd": onesd,
        }
        if with_bias:
            m["bq"] = np.ascontiguousarray(bq_eff[None, QF * t:QF * (t + 1)])
            m["bkv"] = np.ascontiguousarray(bkv_eff[None, :])
            m["bff1"] = np.ascontiguousarray(np.concatenate(
                [bff1_eff[None, FFC * t:FFC * (t + 1)],
                 bff1_eff[None, 2 * FFC + FFC * t:2 * FFC + FFC * (t + 1)]], axis=1))
        in_maps.append(m)

    nc = _get_program(with_bias)
    _CACHED["in_maps"] = in_maps
    res = bass_utils.run_bass_kernel_spmd(nc, in_maps, core_ids=list(range(8)))
    out = np.empty((B, NTOK, DIM), np.float32)
    for s in range(B):
        out[s] = (res.results[2 * s]["out"] + res.results[2 * s + 1]["out"]).T
    return out


# revision 22
# speedup vs baseline: 1.0008x; 1.0008x over previous
"""nn_CrossAttention Trainium2 Bass kernel.

Sharding (8 cores): data-parallel over batch (4 samples x 2 cores) with
2-way Megatron tensor parallelism inside each pair: core = (sample, half).
Each half owns 8 of 16 attention heads (Wq cols / Wout rows) and 2048 of
4096 ff_inner channels (Wff1 cols / Wff2 rows); the tiny shared-head Wkv is
replicated.  Per-core partial outputs (attn@Wout_half + ff@Wff2_half) are
summed pairwise on the host, which also owns the final transpose (the
device computes the output feature-major).

Device kernel (per core, identical SPMD program):
  - LayerNorm token-major via bn_stats (gains folded into the weights on
    the host), then PE-transpose to feature-major.
  - One activation-table set for the whole kernel
    (natural_log_exp_and_others, forced by a post-compile rewrite of the
    InstLoadActFuncSet ids): LN rstd = exp(-0.5*ln(var+eps)), attention
    softmax exp, SwiGLU sigmoid = exp(-ln(1+exp(-g))) all on ScalarE.
  - All matmuls in bf16 with 512-wide moving operands (fp32 PSUM accum).
  - Attention computed transposed (keys/queries feature-major, sim with
    context positions on partitions) so softmax sums fold into the
    attn@v matmul as a ones-column of the [v | 1] stationary operand.
    The two heads of a pair run concurrently on disjoint PE row groups
    (tile_position (0,0)/(64,0)).
  - FF1 (SwiGLU) matmuls interleaved at matmul granularity into the
    attention loops so TensorE never head-of-line blocks on ScalarE exp.
  - Out-projection accumulates the attention and FF paths into one PSUM
    group, streaming Wout/Wff2 once (mt-outer, qc-inner).
"""
import sys

if "/opt/trn_rl_repo" not in sys.path:
    sys.path.insert(0, "/opt/trn_rl_repo")

import numpy as np

import concourse.bass as bass  # noqa: F401  (bass must import before bacc)
import concourse.mybir as mybir
import concourse.tile as tile
from concourse import bacc, bass_utils
from concourse.hw_specs import get_activation_tables

F32 = mybir.dt.float32
BF16 = mybir.dt.bfloat16
AF = mybir.ActivationFunctionType
ALU = mybir.AluOpType

P = 128
B = 4           # batch
NTOK = 1024     # query tokens per sample
NCTX = 1024     # context tokens per sample
DIM = 1024
DH = 64         # head dim
HC = 8          # heads per core (16 total / 2-way TP)
QF = HC * DH    # 512 per-core q features
FFC = 2048      # per-core ff_inner channels
EPS = 1e-5
SCALE = DH ** -0.5

TT = NTOK // P   # 8 token tiles
KT = DIM // P    # 8 contraction tiles over dim
QC = NTOK // 512  # 2 moving-operand chunks of 512 tokens

_CACHED = {}


def _collapse_act_table_loads(nc):
    """All activation funcs used (ln/exp/copy) live in one table set;
    rewrite the compiler-inserted per-function loads into a single load
    of natural_log_exp_and_others at the first site."""
    tables = get_activation_tables(nc.m.arch)
    names = list(tables.keys())
    combined = names.index("natural_log_exp_and_others")
    allowed = tables["natural_log_exp_and_others"]
    used = set()
    for b in nc.main_func.blocks:
        for i in b.instructions:
            if isinstance(i, mybir.InstActivation):
                used.add(i.func)
    assert used <= allowed, f"activation funcs {used - allowed} not in combined set"
    for b in nc.main_func.blocks:
        kept_one = False
        keep = []
        for i in b.instructions:
            if isinstance(i, mybir.InstLoadActFuncSet):
                assert i.sync_info is None
                if not kept_one:
                    i.act_func_set_id = combined
                    kept_one = True
                    keep.append(i)
            else:
                keep.append(i)
        b.instructions[:] = keep


def _build(with_bias: bool):
    nc = bacc.Bacc("TRN2", target_bir_lowering=False, debug=False)

    x_d = nc.dram_tensor("x", [NTOK, DIM], F32, kind="ExternalInput").ap()
    c_d = nc.dram_tensor("ctx", [NCTX, DIM], F32, kind="ExternalInput").ap()
    wq_d = nc.dram_tensor("wq", [DIM, QF], BF16, kind="ExternalInput").ap()
    wkv_d = nc.dram_tensor("wkv", [DIM, 2 * DH], BF16, kind="ExternalInput").ap()
    wout_d = nc.dram_tensor("wout", [QF, DIM], BF16, kind="ExternalInput").ap()
    wff1_d = nc.dram_tensor("wff1", [DIM, 2 * FFC], BF16, kind="ExternalInput").ap()
    wff2_d = nc.dram_tensor("wff2", [FFC, DIM], BF16, kind="ExternalInput").ap()
    eyer_d = nc.dram_tensor("eyer", [P, P], BF16, kind="ExternalInput").ap()
    ones_d = nc.dram_tensor("onesd", [P, 1], BF16, kind="ExternalInput").ap()
    if with_bias:
        bq_d = nc.dram_tensor("bq", [1, QF], F32, kind="ExternalInput").ap()
        bkv_d = nc.dram_tensor("bkv", [1, 2 * DH], F32, kind="ExternalInput").ap()
        bff1_d = nc.dram_tensor("bff1", [1, 2 * FFC], F32, kind="ExternalInput").ap()
    out_d = nc.dram_tensor("out", [DIM, NTOK], F32, kind="ExternalOutput").ap()

    # dram views tiled for lhsT streaming: [p, ktile, cols]
    wq_v = wq_d.rearrange("(ko p) c -> p ko c", p=P)
    wkv_v = wkv_d.rearrange("(ko p) c -> p ko c", p=P)
    wout_v = wout_d.rearrange("(ko p) c -> p ko c", p=P)
    wff1_v = wff1_d.rearrange("(ko p) c -> p ko c", p=P)
    wff2_v = wff2_d.rearrange("(ko p) c -> p ko c", p=P)

    with tile.TileContext(nc) as tc:
        with (
            tc.tile_pool(name="consts", bufs=1) as consts,
            tc.tile_pool(name="lnx", bufs=4) as lnxp,
            tc.tile_pool(name="ln", bufs=3) as lnp,
            tc.tile_pool(name="small", bufs=2) as smallp,
            tc.tile_pool(name="small1", bufs=1) as smallp1,
            tc.tile_pool(name="resid", bufs=1) as resid,
            tc.tile_pool(name="big", bufs=1) as bigp,
            tc.tile_pool(name="wff", bufs=4) as wffp,
            tc.tile_pool(name="wop", bufs=2) as wop,
            tc.tile_pool(name="attn", bufs=2) as attnp,
            tc.tile_pool(name="pm", bufs=3, space="PSUM") as pmp,
            tc.tile_pool(name="po", bufs=2, space="PSUM") as pop,
            tc.tile_pool(name="pf", bufs=3, space="PSUM") as pfp,
        ):
            identr = consts.tile([P, P], BF16)
            nc.sync.dma_start(identr[:], eyer_d[:])
            eps_t = consts.tile([P, 1], F32)
            nc.vector.memset(eps_t[:], EPS)
            one_t = consts.tile([P, 1], F32)
            nc.vector.memset(one_t[:], 1.0)
            # x tile preloads go out on the sync queue before any weight
            # traffic so the LN pipeline starts immediately
            xts = []
            for tt in range(4):
                xt = lnxp.tile([P, DIM], F32, tag="xpre", name="xpre")
                nc.sync.dma_start(xt[:], x_d[tt * P:(tt + 1) * P, :])
                xts.append(xt)
            # HAM warmup: keep the PE busy through the LN head so the first
            # real matmuls run at the full 2.4 GHz clock
            for _ in range(36):
                pw = pmp.tile([P, 512], F32, tag="pm", name="warm")
                nc.tensor.matmul(
                    pw[:, 0:P], identr[:], identr[:], start=True, stop=True
                )
            if with_bias:
                bq_t = consts.tile([P, QF // P], F32)
                nc.sync.dma_start(bq_t[:], bq_d.rearrange("o (fo p) -> p (o fo)", p=P))
                bkv_t = consts.tile([P, 1], F32)
                nc.sync.dma_start(bkv_t[:], bkv_d.rearrange("o (fo p) -> p (o fo)", p=P))
                bff1_t = consts.tile([P, (2 * FFC) // P], F32)
                nc.sync.dma_start(
                    bff1_t[:], bff1_d.rearrange("o (fo p) -> p (o fo)", p=P)
                )

            # persistent activations / weights
            xn_F = resid.tile([P, KT, NTOK], BF16)      # normalized x, feature-major
            cn_F = bigp.tile([P, KT, NCTX], BF16, tag="bigc", name="cn_F")
            qT = resid.tile([P, QF // P, NTOK], BF16)   # queries, feature-major
            kv_sb = resid.tile([P, NCTX], BF16)         # rows 0:64 v, 64:128 k
            kdup = resid.tile([P, NCTX], BF16)          # rows 0:64 = copy of k
            v_aug = resid.tile([P, NCTX // P, DH + 1], BF16)  # [j-in-tile, jt, v|1]
            attn_outT = resid.tile([P, QF // P, NTOK], BF16)
            ff_sc = [
                bigp.tile([P, FFC // P, 512], BF16, tag="big", name="ff_sc0"),
                bigp.tile([P, FFC // P, 512], BF16, tag="big2", name="ff_sc1"),
            ]
            wq_all = resid.tile([P, KT, QF], BF16)
            wkv_t = resid.tile([P, KT, 2 * DH], BF16)
            nc.sync.dma_start(wq_all[:], wq_v[:])
            nc.sync.dma_start(wkv_t[:], wkv_v[:])

            def layernorm_iter(src_dram, dst_fmajor, tt, xt=None):
                if xt is None:
                    xt = lnp.tile([P, DIM], F32, tag="xt", name="xt")
                    nc.sync.dma_start(xt[:], src_dram[tt * P:(tt + 1) * P, :])
                st = lnp.tile([P, 2, nc.vector.BN_STATS_DIM], F32, tag="lnst")
                xv = xt.rearrange("p (s f) -> p s f", s=2)
                nc.vector.bn_stats(st[:, 0, :], xv[:, 0, :])
                nc.vector.bn_stats(st[:, 1, :], xv[:, 1, :])
                mv = lnp.tile([P, nc.vector.BN_AGGR_DIM], F32, tag="lnmv")
                nc.vector.bn_aggr(mv[:], st[:])
                # rstd = exp(-0.5 * ln(var + eps)) — stays in the ln/exp set
                lv = lnp.tile([P, 1], F32, tag="lnlv")
                nc.scalar.activation(
                    out=lv[:], in_=mv[:, 1:2], func=AF.Ln, bias=eps_t[:]
                )
                rstd = lnp.tile([P, 1], F32, tag="lnrs")
                nc.scalar.activation(out=rstd[:], in_=lv[:], func=AF.Exp, scale=-0.5)
                xh = lnp.tile([P, DIM], BF16, tag="lnh")
                nc.vector.tensor_scalar(
                    out=xh[:], in0=xt[:], scalar1=mv[:, 0:1], scalar2=rstd[:],
                    op0=ALU.subtract, op1=ALU.mult,
                )
                for dt_ in range(KT):
                    pt = pmp.tile([P, 512], BF16, tag="pm", name="pt")
                    nc.tensor.transpose(
                        pt[:, 0:P], xh[:, dt_ * P:(dt_ + 1) * P], identr[:]
                    )
                    if dt_ % 2 == 0:
                        nc.vector.tensor_copy(
                            dst_fmajor[:, dt_, tt * P:(tt + 1) * P], pt[:, 0:P]
                        )
                    else:
                        nc.scalar.activation(
                            out=dst_fmajor[:, dt_, tt * P:(tt + 1) * P],
                            in_=pt[:, 0:P], func=AF.Copy,
                        )

            def q_chunk(ft, qc):
                pq = pmp.tile([P, 512], F32, tag="pm")
                for k in range(KT):
                    nc.tensor.matmul(
                        pq[:], wq_all[:, k, ft * P:(ft + 1) * P],
                        xn_F[:, k, qc * 512:(qc + 1) * 512],
                        start=(k == 0), stop=(k == KT - 1),
                    )
                if with_bias:
                    nc.vector.tensor_scalar_add(
                        out=qT[:, ft, qc * 512:(qc + 1) * 512],
                        in0=pq[:], scalar1=bq_t[:, ft:ft + 1],
                    )
                else:
                    nc.vector.tensor_copy(qT[:, ft, qc * 512:(qc + 1) * 512], pq[:])

            # ---- ff1 (SwiGLU) machinery: a chain of matmul-granular steps ----
            ff_sched = [(0, i) for i in range(FFC // P)] + \
                       [(1, i) for i in range(FFC // P)]
            ff_w = {}

            def ff_prefetch(idx):
                if idx >= len(ff_sched) or idx in ff_w:
                    return
                _, i = ff_sched[idx]
                # iters consumed in phases B/C load on the (otherwise idle)
                # scalar HWDGE queue so input loads own the sync queue; the
                # first two stay on sync so their dispatches don't delay the
                # phase-A LN activations queued behind them on ScalarE
                eng = nc.scalar if 2 <= idx < 8 else nc.sync
                wv = wffp.tile([P, KT, P], BF16, tag="ffw", name="ffwv")
                eng.dma_start(wv[:], wff1_v[:, :, i * P:(i + 1) * P])
                wg = wffp.tile([P, KT, P], BF16, tag="ffw", name="ffwg")
                eng.dma_start(
                    wg[:], wff1_v[:, :, FFC + i * P:FFC + (i + 1) * P]
                )
                ff_w[idx] = (wv, wg)

            def ff_ops(idx):
                qc, i = ff_sched[idx]
                wv, wg = ff_w.pop(idx)
                ff_prefetch(idx + 1)
                pv = pfp.tile([P, 512], F32, tag="pf")
                pg = pfp.tile([P, 512], F32, tag="pf")
                for k in range(KT):
                    nc.tensor.matmul(
                        pv[:], wv[:, k, :], xn_F[:, k, qc * 512:(qc + 1) * 512],
                        start=(k == 0), stop=(k == KT - 1),
                    )
                    yield
                for k in range(KT):
                    nc.tensor.matmul(
                        pg[:], wg[:, k, :], xn_F[:, k, qc * 512:(qc + 1) * 512],
                        start=(k == 0), stop=(k == KT - 1),
                    )
                    yield
                # silu(g)*v = g*sigmoid(g)*v; sigmoid via exp/ln/exp only
                if with_bias:
                    gsl = smallp.tile([P, 512], F32, tag="silg")
                    nc.vector.tensor_scalar_add(
                        out=gsl[:], in0=pg[:],
                        scalar1=bff1_t[:, FFC // P + i:FFC // P + i + 1],
                    )
                    vsl = smallp.tile([P, 512], F32, tag="silv")
                    nc.vector.tensor_scalar_add(
                        out=vsl[:], in0=pv[:], scalar1=bff1_t[:, i:i + 1]
                    )
                else:
                    gsl, vsl = pg, pv
                e = smallp.tile([P, 512], F32, tag="sile")
                nc.scalar.activation(out=e[:], in_=gsl[:], func=AF.Exp, scale=-1.0)
                w = smallp.tile([P, 512], F32, tag="silw")
                nc.scalar.activation(out=w[:], in_=e[:], func=AF.Ln, bias=one_t[:])
                sg = smallp.tile([P, 512], F32, tag="sils")
                nc.scalar.activation(out=sg[:], in_=w[:], func=AF.Exp, scale=-1.0)
                m = smallp.tile([P, 512], F32, tag="silm")
                nc.vector.tensor_tensor(m[:], gsl[:], sg[:], ALU.mult)
                nc.vector.tensor_tensor(ff_sc[qc][:, i, :], m[:], vsl[:], ALU.mult)

            def _ff_chain():
                for idx in range(len(ff_sched)):
                    yield from ff_ops(idx)

            ff_chain = _ff_chain()

            def ff_step(n):
                for _ in range(n):
                    next(ff_chain, None)

            # ---- phase A: LN(x) head + weight prefetch ----
            ff_prefetch(0)
            for tt in range(4):
                layernorm_iter(x_d, xn_F, tt, xt=xts[tt])

            # ---- phase B: q qc=0 + early ff1 while LN(x) tail runs on DVE ----
            # x tiles 4/5 ride the first two freed phase-A preload slots so
            # phase B's LN chain isn't exposed to DMA latency either
            xb_pre = {}
            for tt in (4, 5):
                bxt = lnxp.tile([P, DIM], F32, tag="xpre", name="xbpre")
                nc.sync.dma_start(bxt[:], x_d[tt * P:(tt + 1) * P, :])
                xb_pre[tt] = bxt
            ctx_pre = {}

            def ctx_prefetch(ct):
                if ct >= 2 * (QF // P) or ct in ctx_pre:
                    return
                xt = lnp.tile([P, DIM], F32, tag="xt", name="cpre")
                nc.sync.dma_start(xt[:], c_d[ct * P:(ct + 1) * P, :])
                ctx_pre[ct] = xt

            # first two ctx tiles ride the freed phase-A preload slots so
            # phase C's LN chain starts without waiting on the DMA queue
            for ct in range(2):
                cxt = lnxp.tile([P, DIM], F32, tag="xpre", name="cpre")
                nc.scalar.dma_start(cxt[:], c_d[ct * P:(ct + 1) * P, :])
                ctx_pre[ct] = cxt
            for ft in range(QF // P):
                layernorm_iter(x_d, xn_F, 4 + ft, xt=xb_pre.pop(4 + ft, None))
                q_chunk(ft, 0)
                ff_step(8)

            # ---- phase C: q qc=1 + LN(ctx) + early ff1 ----
            for ft in range(QF // P):
                ctx_prefetch(2 * ft + 2)
                layernorm_iter(c_d, cn_F, 2 * ft, xt=ctx_pre.pop(2 * ft))
                q_chunk(ft, 1)
                ctx_prefetch(2 * ft + 3)
                layernorm_iter(c_d, cn_F, 2 * ft + 1, xt=ctx_pre.pop(2 * ft + 1))
                ff_step(16)

            # ---- phase D: kv = cn @ Wkv + v transpose ----
            for jc in range(NCTX // 512):
                pkv = pmp.tile([P, 512], F32, tag="pm")
                for k in range(KT):
                    nc.tensor.matmul(
                        pkv[0:2 * DH, :], wkv_t[:, k, :],
                        cn_F[:, k, jc * 512:(jc + 1) * 512],
                        start=(k == 0), stop=(k == KT - 1),
                    )
                if with_bias:
                    nc.vector.tensor_scalar_add(
                        out=kv_sb[:, jc * 512:(jc + 1) * 512],
                        in0=pkv[0:2 * DH, :], scalar1=bkv_t[:],
                    )
                else:
                    nc.vector.tensor_copy(
                        kv_sb[:, jc * 512:(jc + 1) * 512], pkv[0:2 * DH, :]
                    )
                ff_step(8)
            # k lives at partitions 64:128; duplicate at 0:64 for even heads
            nc.sync.dma_start(kdup[0:DH, :], kv_sb[DH:2 * DH, :])
            # v (partitions 0:64) transposed to token-major with a ones column
            for jt in range(NCTX // P):
                pv_ = pmp.tile([P, 512], BF16, tag="pm")
                nc.tensor.transpose(
                    pv_[:, 0:DH], kv_sb[0:DH, jt * P:(jt + 1) * P],
                    identr[0:DH, 0:DH],
                )
                nc.vector.tensor_copy(v_aug[:, jt, 0:DH], pv_[:, 0:DH])
                ff_step(1)
            nc.sync.dma_start(
                v_aug[:, :, DH:DH + 1],
                bass.AP(tensor=ones_d.tensor, offset=0,
                        ap=[list(ones_d.ap[0]), [0, NCTX // P], list(ones_d.ap[1])]),
            )

            # ---- phase E: attention pairs with ff1 interleaved per-matmul ----
            def attn_pair(ft, qc):
                expT = attnp.tile([P, NCTX // P, 2, 512], BF16, tag="expT")
                qsl = [
                    qT[0:DH, ft, qc * 512:(qc + 1) * 512],
                    qT[DH:2 * DH, ft, qc * 512:(qc + 1) * 512],
                ]
                for jt in range(NCTX // P):
                    ps0 = pmp.tile([P, 512], F32, tag="pm")
                    ps1 = pmp.tile([P, 512], F32, tag="pm")
                    nc.tensor.matmul(
                        ps0[:], kdup[0:DH, jt * P:(jt + 1) * P], qsl[0],
                        start=True, stop=True, tile_position=(0, 0),
                    )
                    nc.tensor.matmul(
                        ps1[:], kv_sb[DH:2 * DH, jt * P:(jt + 1) * P], qsl[1],
                        start=True, stop=True, tile_position=(64, 0),
                    )
                    nc.scalar.activation(out=expT[:, jt, 0, :], in_=ps0[:], func=AF.Exp)
                    nc.scalar.activation(out=expT[:, jt, 1, :], in_=ps1[:], func=AF.Exp)
                    ff_step(4)
                po = [pop.tile([P, 512], F32, tag="po", name=f"po{e}") for e in range(2)]
                for jt in range(NCTX // P):
                    for e in range(2):
                        nc.tensor.matmul(
                            po[e][0:DH + 1, :], v_aug[:, jt, :], expT[:, jt, e, :],
                            start=(jt == 0), stop=(jt == NCTX // P - 1),
                        )
                    ff_step(2)
                for e in range(2):
                    rec = smallp1.tile([P, 512], F32, tag="rec")
                    # move the sums row (psum partition 64) to partition 0
                    nc.vector.tensor_copy(rec[DH:DH + 1, :], po[e][DH:DH + 1, :])
                    nc.sync.dma_start(rec[0:1, :], rec[DH:DH + 1, :])
                    nc.vector.reciprocal_approx_fast(out=rec[0:1, :], in_=rec[0:1, :])
                    rb = smallp1.tile([DH, 512], F32, tag="rb")
                    nc.gpsimd.partition_broadcast(rb[:], rec[0:1, :])
                    if e == 0:
                        nc.vector.tensor_tensor(
                            attn_outT[0:DH, ft, qc * 512:(qc + 1) * 512],
                            po[e][0:DH, :], rb[:], ALU.mult,
                        )
                    else:
                        stg = smallp1.tile([DH, 512], BF16, tag="stg")
                        nc.vector.tensor_tensor(stg[:], po[e][0:DH, :], rb[:], ALU.mult)
                        nc.sync.dma_start(
                            attn_outT[DH:2 * DH, ft, qc * 512:(qc + 1) * 512], stg[:]
                        )

            for ft in range(QF // P):
                for qc in range(QC):
                    attn_pair(ft, qc)

            # ---- phase G setup + phase F: drain remaining ff1 ----
            op_w = {}

            def op_prefetch(mt):
                if mt >= DIM // P or mt in op_w:
                    return
                wo = wop.tile([P, QF // P, P], BF16, tag="wo", name="wo_t")
                nc.sync.dma_start(wo[:], wout_v[:, :, mt * P:(mt + 1) * P])
                wf2 = wop.tile([P, FFC // P, P], BF16, tag="wf2", name="wf2_t")
                nc.sync.dma_start(wf2[:], wff2_v[:, :, mt * P:(mt + 1) * P])
                op_w[mt] = (wo, wf2)

            op_prefetch(0)
            op_prefetch(1)
            ff_step(len(ff_sched) * 2 * KT)
            for mt in range(DIM // P):
                wo, wf2 = op_w.pop(mt)
                op_prefetch(mt + 1)
                for qc in range(QC):
                    pout = pmp.tile([P, 512], F32, tag="pm")
                    for k in range(QF // P):
                        nc.tensor.matmul(
                            pout[:], wo[:, k, :],
                            attn_outT[:, k, qc * 512:(qc + 1) * 512],
                            start=(k == 0), stop=False,
                        )
                    for k in range(FFC // P):
                        nc.tensor.matmul(
                            pout[:], wf2[:, k, :], ff_sc[qc][:, k, :],
                            start=False, stop=(k == FFC // P - 1),
                        )
                    ot = smallp.tile([P, 512], F32, tag="ot")
                    nc.vector.tensor_copy(ot[:], pout[:])
                    nc.sync.dma_start(
                        out_d[mt * P:(mt + 1) * P, qc * 512:(qc + 1) * 512], ot[:]
                    )

    nc.compile()
    _collapse_act_table_loads(nc)
    return nc


def _get_program(with_bias: bool):
    key = ("nc", with_bias)
    if key not in _CACHED:
        _CACHED[key] = _build(with_bias)
    return _CACHED[key]


def kernel(x, context, ln_x_g, ln_x_b, ln_c_g, ln_c_b, Wq, Wkv, Wout, Wff1, Wff2):
    x = np.asarray(x, np.float32)
    context = np.asarray(context, np.float32)
    ln_x_g = np.asarray(ln_x_g, np.float32)
    ln_x_b = np.asarray(ln_x_b, np.float32)
    ln_c_g = np.asarray(ln_c_g, np.float32)
    ln_c_b = np.asarray(ln_c_b, np.float32)
    Wq = np.asarray(Wq, np.float32)
    Wkv = np.asarray(Wkv, np.float32)
    Wout = np.asarray(Wout, np.float32)
    Wff1 = np.asarray(Wff1, np.float32)
    Wff2 = np.asarray(Wff2, np.float32)

    # fold LN gains (and the attention scale) into the weights
    wq_eff = (ln_x_g[:, None] * Wq) * SCALE          # [1024, 1024]
    wkv_eff = ln_c_g[:, None] * Wkv                  # [1024, 128]
    # device kv layout: v at features 0:64, k at 64:128
    wkv_eff = np.concatenate([wkv_eff[:, DH:], wkv_eff[:, :DH]], axis=1)
    wff1_eff = ln_x_g[:, None] * Wff1                # [1024, 8192]
    with_bias = bool(np.any(ln_x_b != 0.0) or np.any(ln_c_b != 0.0))
    if with_bias:
        bq_eff = (ln_x_b @ Wq) * SCALE               # [1024]
        bkv_eff = ln_c_b @ Wkv                       # [128]
        bkv_eff = np.concatenate([bkv_eff[DH:], bkv_eff[:DH]])
        bff1_eff = ln_x_b @ Wff1                     # [8192]

    import ml_dtypes
    bf16 = ml_dtypes.bfloat16
    eye = np.eye(P, dtype=bf16)
    onesd = np.ones((P, 1), bf16)
    in_maps = []
    for c in range(8):
        s, t = c // 2, c % 2
        m = {
            "x": np.ascontiguousarray(x[s]),
            "ctx": np.ascontiguousarray(context[s]),
            "wq": np.ascontiguousarray(wq_eff[:, QF * t:QF * (t + 1)].astype(bf16)),
            "wkv": np.ascontiguousarray(wkv_eff.astype(bf16)),
            "wout": np.ascontiguousarray(Wout[QF * t:QF * (t + 1), :].astype(bf16)),
            "wff1": np.ascontiguousarray(np.concatenate(
                [wff1_eff[:, FFC * t:FFC * (t + 1)],
                 wff1_eff[:, 2 * FFC + FFC * t:2 * FFC + FFC * (t + 1)]],
                axis=1).astype(bf16)),
            "wff2": np.ascontiguousarray(Wff2[FFC * t:FFC * (t + 1), :].astype(bf16)),
            "eyer": eye,
            "onesd": onesd,
        }
        if with_bias:
            m["bq"] = np.ascontiguousarray(bq_eff[None, QF * t:QF * (t + 1)])
            m["bkv"] = np.ascontiguousarray(bkv_eff[None, :])
            m["bff1"] = np.ascontiguousarray(np.concatenate(
                [bff1_eff[None, FFC * t:FFC * (t + 1)],
                 bff1_eff[None, 2 * FFC + FFC * t:2 * FFC + FFC * (t + 1)]], axis=1))
        in_maps.append(m)

    nc = _get_program(with_bias)
    _CACHED["in_maps"] = in_maps
    res = bass_utils.run_bass_kernel_spmd(nc, in_maps, core_ids=list(range(8)))
    out = np.empty((B, NTOK, DIM), np.float32)
    for s in range(B):
        out[s] = (res.results[2 * s]["out"] + res.results[2 * s + 1]["out"]).T
    return out
